# revision 38
# baseline (speedup 1.0000x reference)
"""Trainium2 Bass kernel for 3D-conv attention (4 heads x dim 32, N=4096).

Sharding: one (batch, head) pair per NeuronCore (2 batches x 4 heads = 8 cores).
Host computes the tiny 1x1-conv projections q = (scale*Wq_h)@x, k = Wk_h@x,
v = Wv_h@x in fp32 and ships q/k as fp16 (4x-replicated over partition bands)
and v as fp8-e4m3 in DoubleRow pair layout with a ones column for the row sum.

Per 512-wide query tile, each core computes:
    S^T_c = k_c^T q        j-chunk c on PSUM partitions, queries on free axis;
                           4 chunks run CONCURRENTLY on disjoint 32-row PE
                           bands (q/k replicated at partitions 0/32/64/96).
    es_c = exp(S^T_c - S0) fp8 e4m3, split across BOTH elementwise engines:
                           ACT runs native Exp; DVE runs a Schraudolph
                           bit-trick (round(S*11.54 + b) -> uint8 IS the
                           e4m3 bit pattern of 2^((S-S0)*log2e), with
                           saturating uint8 conversion handling underflow).
    [O;s] += [v|1]^T es    one fp8 DoubleRow matmul per chunk-PAIR
                           (contraction 256 = 2 j-chunks, 0.5 cycles/row),
                           accumulating into PSUM rows 0-47; ones column
                           rides as output row 32.
    evac                   one [33, 512] PSUM->SBUF copy per tile on ACT
                           (the faster exp engine; its slack absorbs the
                           copies), then DMA; host divides by s and applies
                           W_out + bias.

The loop is a flat software pipeline over (tile, group): the PV/evac of tile
t slide under the S/exp of tile t+1, with PV issued in 2-group batches (the
largest burst the 3-pair PSUM backlog can cover without starving the exp
engines) to halve the S<->PV PE-array switch drains. Steady state is a
three-way balance (measured per core: ACT ~73us, DVE ~79us, PE ~78us at the
HAM-throttled 1.2GHz clock) -> ~102.5-103.5us total vs the 162.5us baseline
(occasional runs land ~20% slower when the pod is power-throttled by
outside load).

Hardware facts this design is built on (measured on this pod):
- 4 S matmuls on disjoint 32-row PE bands pack ~4x (tile_position row tiling);
- fp8 DoubleRow halves PV matmul cycles (contraction 256/pair), beating
  fp16 2x-column-tiled PV despite its serial LDWEIGHTS;
- DVE fp32->uint8 conversion is round-to-nearest with saturation, making the
  one-instruction Schraudolph exp bit-trick exact enough (rel err 1.1e-2
  end-to-end vs the 2e-2 gate);
- exp from PSUM runs at 1x on both engines (no 2x modes), so splitting
  chunk-pairs ACT/DVE per group is the only way to scale exp throughput.
"""

import numpy as np
import ml_dtypes

import concourse.bass as bass
import concourse.tile as tile
from concourse import bacc, mybir
from concourse.bass_utils import run_bass_kernel_spmd

HEADS = 4
DH = 32
DIM = 128
N = 4096
TI = 512            # i-tile (query) width = one PSUM bank of fp32
NT = N // TI        # 8 i-tiles
CH = 128            # j-chunk width = PE partition count
NCH = N // CH       # 32 chunks
NPAIR = NCH // 2    # 16 chunk-pairs (exp + DoubleRow granularity)
MV = DH + 1         # live PV output rows: 32 v-dims + ones row
MP = 48             # padded DoubleRow stationary width (step%16==0)

S0 = 2.5            # global exp shift: es = exp(S - S0); S in [-8.5, 7.8]
A8 = 11.5416        # 8*log2(e)
B8 = 56.0 - S0 * A8 - 0.45   # e4m3 exp bias + minimax sigma

F32 = mybir.dt.float32
F16 = mybir.dt.float16
F8 = mybir.dt.float8e4
U8 = mybir.dt.uint8
EXP = mybir.ActivationFunctionType.Exp
DR = mybir.MatmulPerfMode.DoubleRow

N_CORES = 8
_np_f16 = np.float16
_np_f8 = ml_dtypes.float8_e4m3

LAST_RESULTS = None  # BassKernelResults of the most recent run (for test harness)
TRACE = False


def _ensure_ntff_hook():
    """Make ``antenv.axon_hooks`` importable so trace-enabled runs work (or
    degrade gracefully)."""
    try:
        import antenv.axon_hooks  # noqa: F401
        return True
    except ImportError:
        pass
    import sys
    import types
    hook = None
    try:
        from trn_agent_boot.trn_boot import _ntff_profile_via_ctypes
        hook = _ntff_profile_via_ctypes("/opt/axon/libaxon_pjrt.so")
    except Exception:
        pass
    try:
        import antenv
        mod = types.ModuleType("antenv.axon_hooks")
        state = {"hook": hook}
        mod.get_axon_ntff_profile_hook = lambda: state["hook"]
        mod.set_axon_ntff_profile_hook = lambda h: state.update(hook=h)
        sys.modules["antenv.axon_hooks"] = mod
        antenv.axon_hooks = mod
    except Exception as e:  # pragma: no cover
        print(f"ntff hook setup failed ({e}); running without trace")
        return False
    return hook is not None


def build_nc():
    nc = bacc.Bacc(None)
    q_d = nc.dram_tensor("q4", [DIM, N], F16, kind="ExternalInput")
    k_d = nc.dram_tensor("k4", [DIM, N], F16, kind="ExternalInput")
    vt_d = nc.dram_tensor("vt", [DIM, NPAIR, 2, MP], F8, kind="ExternalInput")
    o_d = nc.dram_tensor("o", [NT, MV, TI], F32, kind="ExternalOutput")

    with tile.TileContext(nc) as tc:
        with (
            tc.tile_pool(name="singles", bufs=1) as singles,
            tc.tile_pool(name="ep", bufs=12) as ep,
            tc.tile_pool(name="outp", bufs=4) as outp,
            tc.tile_pool(name="psS", bufs=3, space="PSUM") as psS,
            tc.tile_pool(name="psO", bufs=1, space="PSUM") as psO,
        ):
            q_sb = singles.tile([DIM, N], F16)
            k_sb = singles.tile([DIM, N], F16)
            vT = singles.tile([DIM, NPAIR, 2, MP], F8)
            bias_sb = singles.tile([DIM, 1], F32)
            nc.vector.memset(bias_sb[:], -S0)

            # First S chunk needs k[:, 0:256] and q tile 0; stream those first.
            nc.sync.dma_start(out=k_sb[:, 0:256], in_=k_d[:, 0:256])
            nc.sync.dma_start(out=q_sb[:, 0:TI], in_=q_d[:, 0:TI])
            nc.sync.dma_start(out=k_sb[:, 256:TI], in_=k_d[:, 256:TI])
            nc.sync.dma_start(out=k_sb[:, TI : N // 2], in_=k_d[:, TI : N // 2])
            nc.sync.dma_start(out=k_sb[:, N // 2 : N], in_=k_d[:, N // 2 : N])
            nc.sync.dma_start(out=vT[:], in_=vt_d[:])
            nc.sync.dma_start(out=q_sb[:, TI:N], in_=q_d[:, TI:N])

            NG = NPAIR // 2  # 8 groups of 2 pairs per i-tile
            # Flat pipeline over (tile, group): PV and evacuation of tile t
            # slide under the S/exp of tile t+1 (PV lags one group globally).
            pO_tiles = {}
            es_tiles = {}

            def pv_group(t, g):
                if t < 0:
                    return
                if g == 0:
                    pO_tiles[t] = psO.tile([MP, TI], F32, name=f"pO{t % 2}")
                pO = pO_tiles[t]
                for p in (2 * g, 2 * g + 1):
                    nc.tensor.matmul(
                        pO[:], vT[:, p, :, :], es_tiles[(t, p)][:],
                        start=(p == 0), stop=(p == NPAIR - 1),
                        perf_mode=DR,
                        skip_group_check=True,
                    )
                    del es_tiles[(t, p)]
                if g == NG - 1:
                    # evac on ACT always: ACT is the faster exp engine and
                    # has ~5us of slack vs DVE, so this balances both.
                    ov = outp.tile([MV, TI], F32, tag="ov")
                    nc.scalar.copy(ov[:], pO[0:MV, :])
                    nc.sync.dma_start(out=o_d[t], in_=ov[:])
                    del pO_tiles[t]

            for step in range(NT * NG):
                t, g = divmod(step, NG)
                # 4 S matmuls on bands 0-3 back-to-back: they pack on
                # disjoint PE row groups (one wave ~= one matmul time).
                pS2 = []
                for h in range(2):
                    pS = psS.tile([DIM, 2, TI], F32, tag="ps")
                    for j in range(2):
                        c = 4 * g + 2 * h + j
                        b = c % 4
                        nc.tensor.matmul(
                            pS[:, j, :],
                            k_sb[bass.ds(32 * b, DH), bass.ts(c, CH)],
                            q_sb[bass.ds(32 * b, DH), bass.ts(t, TI)],
                            start=True, stop=True,
                            tile_position=(32 * b, 0),
                        )
                    pS2.append(pS)
                # exp: ACT takes pair A, DVE pair B -- concurrently. ACT must
                # keep pair A: PV consumes pairs in order, so the faster
                # engine finishing the first-needed pair keeps the PE fed
                # (swapping measured +6us). Giving ACT extra pairs
                # double-books a group and destabilizes the pipeline
                # (measured +20us), so the split stays strictly 1:1 and
                # ACT's slack absorbs the evacs.
                for h in range(2):
                    p = 2 * g + h
                    e_t = ep.tile([DIM, 2, TI], F8, tag="es")
                    # step 0 only: ACT takes both pairs -- during warmup DVE
                    # is still waiting on the k DMA, so this fills its stall
                    # without perturbing the steady-state phase.
                    if h == 0 or step == 0:
                        nc.scalar.activation(e_t[:], pS2[h][:], func=EXP,
                                             bias=bias_sb[:])
                    else:
                        nc.vector.tensor_scalar(
                            out=e_t[:].bitcast(U8), in0=pS2[h][:],
                            scalar1=A8, scalar2=B8,
                            op0=mybir.AluOpType.mult,
                            op1=mybir.AluOpType.add)
                    es_tiles[(t, p)] = e_t
                # lagged DoubleRow PV, issued in 2-group batches (4 matmuls)
                # every other step: halves the number of S<->PV PE array
                # switch drains (S waves and PV matmuls share sub-arrays, so
                # each transition costs a pipeline drain).
                if step % 2 == 1:
                    for back in (2, 1):
                        if step - back >= 0:
                            pt, pg = divmod(step - back, NG)
                            pv_group(pt, pg)
            pv_group(NT - 1, NG - 1)
    nc.compile()
    return nc


def kernel(input, w_qkv, w_out, b_out):
    global LAST_RESULTS
    input = np.asarray(input, dtype=np.float32)
    w_qkv = np.asarray(w_qkv, dtype=np.float32)
    w_out = np.asarray(w_out, dtype=np.float32)
    b_out = np.asarray(b_out, dtype=np.float32)

    b, c, X, Y, Z = input.shape
    n = X * Y * Z
    assert (b, c, n) == (2, DIM, N), (b, c, n)
    xf = input.reshape(b, c, n)
    scale = DH ** -0.5
    hid = HEADS * DH

    in_maps = []
    wos = []
    for core in range(N_CORES):
        bi, h = divmod(core, HEADS)
        wq = w_qkv[h * DH : (h + 1) * DH, :] * scale
        wk = w_qkv[hid + h * DH : hid + (h + 1) * DH, :]
        wv = w_qkv[2 * hid + h * DH : 2 * hid + (h + 1) * DH, :]
        wos.append(w_out[:, h * DH : (h + 1) * DH])
        xb = xf[bi]
        q = (wq @ xb).astype(_np_f16)            # [32, N]
        k = (wk @ xb).astype(_np_f16)
        v = (wv @ xb).astype(np.float32)         # [32, N]
        vt = np.zeros((DIM, NPAIR, 2, MP), _np_f8)
        # vt[p, pair, kt, m] = v[m, pair*256 + kt*128 + p]
        vt[:, :, :, 0:DH] = v.astype(_np_f8).reshape(
            DH, NPAIR, 2, CH).transpose(3, 1, 2, 0)
        vt[:, :, :, DH] = 1.0
        in_maps.append({
            "q4": np.ascontiguousarray(np.tile(q, (4, 1))),
            "k4": np.ascontiguousarray(np.tile(k, (4, 1))),
            "vt": vt,
        })

    nc = build_nc()
    hook_ok = _ensure_ntff_hook()
    LAST_RESULTS = run_bass_kernel_spmd(nc, in_maps, list(range(N_CORES)),
                                        trace=TRACE and hook_ok)
    results = LAST_RESULTS.results

    out = np.zeros((b, c, n), np.float32)
    for core in range(N_CORES):
        bi, _ = divmod(core, HEADS)
        o = results[core]["o"]                     # [NT, 33, TI]
        O = o.transpose(1, 0, 2).reshape(MV, N)    # [33, N]
        attn = O[0:DH] / O[DH]                     # [32, N]
        out[bi] += wos[core] @ attn
    out += b_out[None, :, None]
    return out.reshape(b, c, X, Y, Z)


# revision 40
# speedup vs baseline: 1.3495x; 1.3495x over previous
"""Trainium2 Bass kernel for 3D-conv attention (4 heads x dim 32, N=4096).

Sharding: one (batch, head) pair per NeuronCore (2 batches x 4 heads = 8 cores).
Host computes the tiny 1x1-conv projections q = (scale*Wq_h)@x, k = Wk_h@x,
v = Wv_h@x in fp32 and ships q/k as fp16 (4x-replicated over partition bands)
and v as fp8-e4m3 in DoubleRow pair layout with a ones column for the row sum.

Per 512-wide query tile, each core computes:
    S^T_c = k_c^T q        j-chunk c on PSUM partitions, queries on free axis;
                           4 chunks run CONCURRENTLY on disjoint 32-row PE
                           bands (q/k replicated at partitions 0/32/64/96).
    es_c = exp(S^T_c - S0) fp8 e4m3, split across BOTH elementwise engines:
                           ACT runs native Exp; DVE runs a Schraudolph
                           bit-trick (round(S*11.54 + b) -> uint8 IS the
                           e4m3 bit pattern of 2^((S-S0)*log2e), with
                           saturating uint8 conversion handling underflow).
    [O;s] += [v|1]^T es    one fp8 DoubleRow matmul per chunk-PAIR
                           (contraction 256 = 2 j-chunks, 0.5 cycles/row),
                           accumulating into PSUM rows 0-47; ones column
                           rides as output row 32.
    evac                   one [33, 512] PSUM->SBUF copy per tile on ACT
                           (the faster exp engine; its slack absorbs the
                           copies), then DMA; host divides by s and applies
                           W_out + bias.

The loop is a flat software pipeline over (tile, group): the PV/evac of tile
t slide under the S/exp of tile t+1, with PV issued in 2-group batches (the
largest burst the 3-pair PSUM backlog can cover without starving the exp
engines) to halve the S<->PV PE-array switch drains. Steady state is a
three-way balance (measured per core: ACT ~73us, DVE ~79us, PE ~78us at the
HAM-throttled 1.2GHz clock) -> ~102.5-103.5us total vs the 162.5us baseline
(occasional runs land ~20% slower when the pod is power-throttled by
outside load).

Hardware facts this design is built on (measured on this pod):
- 4 S matmuls on disjoint 32-row PE bands pack ~4x (tile_position row tiling);
- fp8 DoubleRow halves PV matmul cycles (contraction 256/pair), beating
  fp16 2x-column-tiled PV despite its serial LDWEIGHTS;
- DVE fp32->uint8 conversion is round-to-nearest with saturation, making the
  one-instruction Schraudolph exp bit-trick exact enough (rel err 1.1e-2
  end-to-end vs the 2e-2 gate);
- exp from PSUM runs at 1x on both engines (no 2x modes), so splitting
  chunk-pairs ACT/DVE per group is the only way to scale exp throughput.
"""

import numpy as np
import ml_dtypes

import concourse.bass as bass
import concourse.tile as tile
from concourse import bacc, mybir
from concourse.bass_utils import run_bass_kernel_spmd

HEADS = 4
DH = 32
DIM = 128
N = 4096
TI = 512            # i-tile (query) width = one PSUM bank of fp32
NT = N // TI        # 8 i-tiles
CH = 128            # j-chunk width = PE partition count
NCH = N // CH       # 32 chunks
NPAIR = NCH // 2    # 16 chunk-pairs (exp + DoubleRow granularity)
MV = DH + 1         # live PV output rows: 32 v-dims + ones row
MP = 48             # padded DoubleRow stationary width (step%16==0)

S0 = 2.5            # global exp shift: es = exp(S - S0); S in [-8.5, 7.8]
A8 = 11.5416        # 8*log2(e)
B8 = 56.0 - S0 * A8 - 0.45   # e4m3 exp bias + minimax sigma

F32 = mybir.dt.float32
F16 = mybir.dt.float16
F8 = mybir.dt.float8e4
U8 = mybir.dt.uint8
EXP = mybir.ActivationFunctionType.Exp
DR = mybir.MatmulPerfMode.DoubleRow

N_CORES = 8
_np_f16 = np.float16
_np_f8 = ml_dtypes.float8_e4m3

LAST_RESULTS = None  # BassKernelResults of the most recent run (for test harness)
TRACE = False


def _ensure_ntff_hook():
    """Make ``antenv.axon_hooks`` importable so trace-enabled runs work (or
    degrade gracefully)."""
    try:
        import antenv.axon_hooks  # noqa: F401
        return True
    except ImportError:
        pass
    import sys
    import types
    hook = None
    try:
        from trn_agent_boot.trn_boot import _ntff_profile_via_ctypes
        hook = _ntff_profile_via_ctypes("/opt/axon/libaxon_pjrt.so")
    except Exception:
        pass
    try:
        import antenv
        mod = types.ModuleType("antenv.axon_hooks")
        state = {"hook": hook}
        mod.get_axon_ntff_profile_hook = lambda: state["hook"]
        mod.set_axon_ntff_profile_hook = lambda h: state.update(hook=h)
        sys.modules["antenv.axon_hooks"] = mod
        antenv.axon_hooks = mod
    except Exception as e:  # pragma: no cover
        print(f"ntff hook setup failed ({e}); running without trace")
        return False
    return hook is not None


def build_nc():
    nc = bacc.Bacc(None)
    q_d = nc.dram_tensor("q4", [DIM, N], F16, kind="ExternalInput")
    k_d = nc.dram_tensor("k4", [DIM, N], F16, kind="ExternalInput")
    vt_d = nc.dram_tensor("vt", [DIM, NPAIR, 2, MP], F8, kind="ExternalInput")
    o_d = nc.dram_tensor("o", [NT, MV, TI], F32, kind="ExternalOutput")

    with tile.TileContext(nc) as tc:
        with (
            tc.tile_pool(name="singles", bufs=1) as singles,
            tc.tile_pool(name="ep", bufs=12) as ep,
            tc.tile_pool(name="outp", bufs=4) as outp,
            tc.tile_pool(name="psS", bufs=3, space="PSUM") as psS,
            tc.tile_pool(name="psO", bufs=1, space="PSUM") as psO,
        ):
            q_sb = singles.tile([DIM, N], F16)
            k_sb = singles.tile([DIM, N], F16)
            vT = singles.tile([DIM, NPAIR, 2, MP], F8)
            bias_sb = singles.tile([DIM, 1], F32)
            nc.vector.memset(bias_sb[:], -S0)

            # First S chunk needs k[:, 0:256] and q tile 0; stream those first.
            nc.sync.dma_start(out=k_sb[:, 0:256], in_=k_d[:, 0:256])
            nc.sync.dma_start(out=q_sb[:, 0:TI], in_=q_d[:, 0:TI])
            nc.sync.dma_start(out=k_sb[:, 256:TI], in_=k_d[:, 256:TI])
            nc.sync.dma_start(out=k_sb[:, TI : N // 2], in_=k_d[:, TI : N // 2])
            nc.sync.dma_start(out=k_sb[:, N // 2 : N], in_=k_d[:, N // 2 : N])
            nc.sync.dma_start(out=vT[:], in_=vt_d[:])
            nc.sync.dma_start(out=q_sb[:, TI:N], in_=q_d[:, TI:N])

            NG = NPAIR // 2  # 8 groups of 2 pairs per i-tile
            # Flat pipeline over (tile, group): PV and evacuation of tile t
            # slide under the S/exp of tile t+1 (PV lags one group globally).
            pO_tiles = {}
            es_tiles = {}

            def pv_group(t, g):
                if t < 0:
                    return
                if g == 0:
                    pO_tiles[t] = psO.tile([MP, TI], F32, name=f"pO{t % 2}")
                pO = pO_tiles[t]
                for p in (2 * g, 2 * g + 1):
                    nc.tensor.matmul(
                        pO[:], vT[:, p, :, :], es_tiles[(t, p)][:],
                        start=(p == 0), stop=(p == NPAIR - 1),
                        perf_mode=DR,
                        skip_group_check=True,
                    )
                    del es_tiles[(t, p)]
                if g == NG - 1:
                    # evac on ACT always: ACT is the faster exp engine and
                    # has ~5us of slack vs DVE, so this balances both.
                    ov = outp.tile([MV, TI], F32, tag="ov")
                    nc.scalar.copy(ov[:], pO[0:MV, :])
                    nc.sync.dma_start(out=o_d[t], in_=ov[:])
                    del pO_tiles[t]

            for step in range(NT * NG):
                t, g = divmod(step, NG)
                # 4 S matmuls on bands 0-3 back-to-back: they pack on
                # disjoint PE row groups (one wave ~= one matmul time).
                pS2 = []
                for h in range(2):
                    pS = psS.tile([DIM, 2, TI], F32, tag="ps")
                    for j in range(2):
                        c = 4 * g + 2 * h + j
                        b = c % 4
                        nc.tensor.matmul(
                            pS[:, j, :],
                            k_sb[bass.ds(32 * b, DH), bass.ts(c, CH)],
                            q_sb[bass.ds(32 * b, DH), bass.ts(t, TI)],
                            start=True, stop=True,
                            tile_position=(32 * b, 0),
                        )
                    pS2.append(pS)
                # exp: ACT takes pair A, DVE pair B -- concurrently. ACT must
                # keep pair A: PV consumes pairs in order, so the faster
                # engine finishing the first-needed pair keeps the PE fed
                # (swapping measured +6us). Giving ACT extra pairs
                # double-books a group and destabilizes the pipeline
                # (measured +20us), so the split stays strictly 1:1 and
                # ACT's slack absorbs the evacs.
                for h in range(2):
                    p = 2 * g + h
                    e_t = ep.tile([DIM, 2, TI], F8, tag="es")
                    if h == 0:
                        nc.scalar.activation(e_t[:], pS2[h][:], func=EXP,
                                             bias=bias_sb[:])
                    else:
                        nc.vector.tensor_scalar(
                            out=e_t[:].bitcast(U8), in0=pS2[h][:],
                            scalar1=A8, scalar2=B8,
                            op0=mybir.AluOpType.mult,
                            op1=mybir.AluOpType.add)
                    es_tiles[(t, p)] = e_t
                # lagged DoubleRow PV, issued in 2-group batches (4 matmuls)
                # every other step: halves the number of S<->PV PE array
                # switch drains (S waves and PV matmuls share sub-arrays, so
                # each transition costs a pipeline drain).
                if step % 2 == 1:
                    for back in (2, 1):
                        if step - back >= 0:
                            pt, pg = divmod(step - back, NG)
                            pv_group(pt, pg)
            pv_group(NT - 1, NG - 1)
    nc.compile()
    return nc


def kernel(input, w_qkv, w_out, b_out):
    global LAST_RESULTS
    input = np.asarray(input, dtype=np.float32)
    w_qkv = np.asarray(w_qkv, dtype=np.float32)
    w_out = np.asarray(w_out, dtype=np.float32)
    b_out = np.asarray(b_out, dtype=np.float32)

    b, c, X, Y, Z = input.shape
    n = X * Y * Z
    assert (b, c, n) == (2, DIM, N), (b, c, n)
    xf = input.reshape(b, c, n)
    scale = DH ** -0.5
    hid = HEADS * DH

    in_maps = []
    wos = []
    for core in range(N_CORES):
        bi, h = divmod(core, HEADS)
        wq = w_qkv[h * DH : (h + 1) * DH, :] * scale
        wk = w_qkv[hid + h * DH : hid + (h + 1) * DH, :]
        wv = w_qkv[2 * hid + h * DH : 2 * hid + (h + 1) * DH, :]
        wos.append(w_out[:, h * DH : (h + 1) * DH])
        xb = xf[bi]
        q = (wq @ xb).astype(_np_f16)            # [32, N]
        k = (wk @ xb).astype(_np_f16)
        v = (wv @ xb).astype(np.float32)         # [32, N]
        vt = np.zeros((DIM, NPAIR, 2, MP), _np_f8)
        # vt[p, pair, kt, m] = v[m, pair*256 + kt*128 + p]
        vt[:, :, :, 0:DH] = v.astype(_np_f8).reshape(
            DH, NPAIR, 2, CH).transpose(3, 1, 2, 0)
        vt[:, :, :, DH] = 1.0
        in_maps.append({
            "q4": np.ascontiguousarray(np.tile(q, (4, 1))),
            "k4": np.ascontiguousarray(np.tile(k, (4, 1))),
            "vt": vt,
        })

    nc = build_nc()
    hook_ok = _ensure_ntff_hook()
    LAST_RESULTS = run_bass_kernel_spmd(nc, in_maps, list(range(N_CORES)),
                                        trace=TRACE and hook_ok)
    results = LAST_RESULTS.results
    # Rare transient device glitches can return zeroed/garbage tiles (seen
    # once in ~37 runs as a NaN output during a heavily-throttled window).
    # The denominator row is >= exp(-S0-8.5) > 0 for real data, so any
    # non-finite or non-positive s marks a bad run: re-execute once.
    def _bad(rs):
        return any(
            (not np.isfinite(r["o"]).all()) or (r["o"][:, DH, :] <= 0).any()
            for r in rs
        )
    if _bad(results):
        LAST_RESULTS = run_bass_kernel_spmd(nc, in_maps, list(range(N_CORES)),
                                            trace=False)
        results = LAST_RESULTS.results

    out = np.zeros((b, c, n), np.float32)
    for core in range(N_CORES):
        bi, _ = divmod(core, HEADS)
        o = results[core]["o"]                     # [NT, 33, TI]
        O = o.transpose(1, 0, 2).reshape(MV, N)    # [33, N]
        attn = O[0:DH] / O[DH]                     # [32, N]
        out[bi] += wos[core] @ attn
    out += b_out[None, :, None]
    return out.reshape(b, c, X, Y, Z)
